# revision 43
# baseline (speedup 1.0000x reference)
"""Trainium2 Bass kernel for nn_MoEAdapterBranch.

Reference computation (B=16, T=1025, D=1024, E=8, R=128, C=192):
  cls, h = x[:, :1], x[:, 1:]
  z = h @ Q;  e = z^2 / (sum_r z^2 + 1e-6)
  m = softmax(masks, axis=0);  logits = (e * gamma) @ m.T + bias
  probs = softmax(logits / tau)
  mid = gelu(h @ W_down[e].T);  out = sum_e probs_e * (mid_e @ W_up[e].T)
  y = h + alpha * out;  full = concat(cls, y)
  plus scalar outputs: ortho penalty (Q,P) and routing entropy.

Strategy: data-parallel over batch across 8 NeuronCores (2 batches = 2048
tokens per core), weights replicated, everything computed in a transposed
"feature-on-partition / token-on-free" layout so no on-device transposes are
needed.  Host pre-transposes x and the weights (part of shard prep) and
re-transposes the output.  Matmuls run in float32r (TF32-like, 1 cyc/row at
N>=256 vs 4 cyc/row for fp32).  The expert loop is folded into single big
stacked matmuls: down = [D,E*C] weight, up = [E*C,D] weight with the
probability weighting applied to the gelu activations (equivalent to the
probability-weighted combine).  Router and adapter stages are interleaved
(software pipelined by the Tile scheduler) so the PE stays busy through the
router's ACT/DVE latency chains; entropy (the only Ln user) is batched at the
end to minimize ACT table loads.
"""

import sys

sys.path.insert(0, "/opt/trn_rl_repo")

import numpy as np

import concourse.bass as bass
import concourse.tile as tile
from concourse import bacc, mybir
from concourse.bass_utils import run_bass_kernel_spmd

# problem shapes (hardcoded per contract)
B, T, D = 16, 1025, 1024
E, R, C = 8, 128, 192
TAU = 1.0
ORTHO_LAMBDA = 1e-3

NCORES = 8
BPC = B // NCORES          # batches per core
TOK = BPC * (T - 1)        # tokens per core (2048)
TT = 512                   # token tile (fp32 moving-operand max)
NT = TOK // TT             # token tiles per core (4)
DK = D // 128              # contraction chunks over D (8)
EC = E * C                 # stacked expert-channel dim (1536)
MC = EC // 128             # stacked chunks (12)

F32 = mybir.dt.float32
F32R = mybir.dt.float32r
AF = mybir.ActivationFunctionType
ALU = mybir.AluOpType
AX = mybir.AxisListType


def _r(ap):
    return ap.bitcast(F32R)


def _build():
    nc = bacc.Bacc("TRN2", target_bir_lowering=False, debug=False, num_devices=NCORES)

    ht_d = nc.dram_tensor("ht", [D, TOK], F32, kind="ExternalInput").ap()
    wd_d = nc.dram_tensor("wd", [D, EC], F32, kind="ExternalInput").ap()
    wu_d = nc.dram_tensor("wu", [EC, D], F32, kind="ExternalInput").ap()
    q_d = nc.dram_tensor("q", [D, R], F32, kind="ExternalInput").ap()
    p_d = nc.dram_tensor("p", [D, R], F32, kind="ExternalInput").ap()
    mt_d = nc.dram_tensor("mt", [R, E], F32, kind="ExternalInput").ap()
    gam_d = nc.dram_tensor("gam", [R, 1], F32, kind="ExternalInput").ap()
    bia_d = nc.dram_tensor("bia", [E, 1], F32, kind="ExternalInput").ap()
    sel_d = nc.dram_tensor("sel", [32, EC], F32, kind="ExternalInput").ap()
    onep_d = nc.dram_tensor("onep", [128, 128], F32, kind="ExternalInput").ap()
    one32_d = nc.dram_tensor("one32", [32, E], F32, kind="ExternalInput").ap()
    zed_d = nc.dram_tensor("zed", [1, TT], F32, kind="ExternalInput").ap()
    eye_d = nc.dram_tensor("eye", [128, 128], F32, kind="ExternalInput").ap()

    yt_d = nc.dram_tensor("yt", [D, TOK], F32, kind="ExternalOutput").ap()
    ent_d = nc.dram_tensor("ent", [E, NT], F32, kind="ExternalOutput").ap()
    orth_d = nc.dram_tensor("orth", [128, 2], F32, kind="ExternalOutput").ap()

    with tile.TileContext(nc) as tc:
        with (
            tc.tile_pool(name="const", bufs=1) as const,
            tc.tile_pool(name="ht", bufs=3) as htp,
            tc.tile_pool(name="rt", bufs=1) as rt,
            tc.tile_pool(name="rt8", bufs=1) as rt8,
            tc.tile_pool(name="mid", bufs=1) as midp,
            tc.tile_pool(name="y", bufs=2) as yp,
            tc.tile_pool(name="ps", bufs=8, space="PSUM") as ps,
        ):
            # ---- q + first h tile get DMA-queue priority -----------------
            q_sb = const.tile([128, DK, R], F32, name="q")
            nc.sync.dma_start(_r(q_sb[:]), _r(q_d.rearrange("(k p) n -> p k n", p=128)))

            def consts():
                out = {}
                out["mt"] = mt_sb = const.tile([R, E], F32, name="mt")
                nc.sync.dma_start(mt_sb[:], mt_d[:])
                out["gam"] = gam_sb = const.tile([R, 1], F32, name="gam")
                nc.sync.dma_start(gam_sb[:], gam_d[:])
                out["bia"] = bia_sb = const.tile([E, 1], F32, name="bia")
                nc.sync.dma_start(bia_sb[:], bia_d[:])
                out["sel"] = sel_sb = const.tile([32, EC], F32, name="sel")
                nc.scalar.dma_start(_r(sel_sb[:]), _r(sel_d[:]))
                out["onesP"] = onesP = const.tile([128, 128], F32, name="onesP")
                nc.sync.dma_start(_r(onesP[:]), _r(onep_d[:]))
                out["ones32"] = ones32 = const.tile([32, E], F32, name="ones32")
                nc.sync.dma_start(_r(ones32[:]), _r(one32_d[:]))
                out["probs"] = probs_all = const.tile([32, NT, TT], F32, name="probs")
                # zero only the pad rows 8:32 (the router writes rows 0:8, so
                # no WAW between the zero-fill and per-tile probs writes)
                zbc = bass.AP(tensor=zed_d.tensor, offset=0, ap=[[0, 24], [0, NT], [1, TT]])
                nc.sync.dma_start(_r(probs_all[E:, :, :]), _r(zbc))
                out["eye"] = eye_sb = const.tile([128, 128], F32, name="eye")
                nc.sync.dma_start(eye_sb[:], eye_d[:])
                return out

            # ---- big weight tiles (DMAs emitted inside the schedule) -----
            wd_sb = const.tile([128, DK, EC], F32, name="wd")
            wu_sb = const.tile([128, MC, D], F32, name="wu")

            def dma_wd():
                # ACT is idle before the first gelu, so it can issue these
                # without queueing behind compute (both SP and ACT are HWDGE)
                for g in range(MC):
                    eng = nc.scalar if g % 2 == 0 else nc.sync
                    eng.dma_start(
                        _r(wd_sb[:, :, bass.ts(g, 128)]),
                        _r(wd_d[:, bass.ts(g, 128)].rearrange("(k p) n -> p k n", p=128)),
                    )

            def dma_wu():
                for g in range(DK):
                    eng = nc.scalar if g % 2 == 0 else nc.sync
                    eng.dma_start(
                        _r(wu_sb[:, :, bass.ts(g, 128)]),
                        _r(wu_d[:, bass.ts(g, 128)].rearrange("(k p) n -> p k n", p=128)),
                    )

            ent_sb = const.tile([E, NT], F32, name="entsb")
            orth_sb = const.tile([128, 2], F32, name="orthsb")

            def mask_softmax(cb):
                mt_sb, gam_sb = cb["mt"], cb["gam"]
                neg_mx = const.tile([R, 1], F32, name="negmx")
                nc.vector.tensor_reduce(neg_mx[:], mt_sb[:], axis=AX.X, op=ALU.max, negate=True)
                me = const.tile([R, E], F32, name="me")
                nc.scalar.activation(me[:], mt_sb[:], AF.Exp, bias=neg_mx[:], scale=1.0)
                msum = const.tile([R, 1], F32, name="msum")
                nc.vector.tensor_reduce(msum[:], me[:], axis=AX.X, op=ALU.add)
                minv = const.tile([R, 1], F32, name="minv")
                nc.vector.reciprocal(minv[:], msum[:])
                m_sm = const.tile([R, E], F32, name="msm")
                nc.vector.tensor_scalar_mul(m_sm[:], me[:], minv[:])
                gm = const.tile([R, E], F32, name="gmt")
                nc.vector.tensor_scalar_mul(_r(gm[:]), m_sm[:], gam_sb[:])
                return gm

            ht_tiles = {}
            expl_tiles = {}

            def r_front(t, pre_ht=None):
                # z = Q^T h; z2 = z^2; rsum = sum_r z2 (replicated);
                # logits_raw = gm^T z2; logits = logits_raw / rsum  (the
                # 1/(sum+1e-6) normalization commutes with the R-contraction;
                # 1e-6 vs sum~128 is below fp32 ulp); expl = exp(logits+bias)
                if pre_ht is None:
                    ht = htp.tile([128, DK, TT], F32, tag="ht")
                    nc.sync.dma_start(
                        _r(ht[:]),
                        _r(ht_d[:, bass.ts(t, TT)].rearrange("(k p) n -> p k n", p=128)),
                    )
                else:
                    ht = pre_ht
                ht_tiles[t] = ht
                zp = ps.tile([128, TT], F32, tag="bank", name=f"z{t}")
                for k in range(DK):
                    nc.tensor.matmul(
                        zp[:], _r(q_sb[:, k, :]), _r(ht[:, k, :]),
                        start=(k == 0), stop=(k == DK - 1),
                    )
                z2 = rt.tile([128, TT], F32, tag="z2")
                nc.scalar.activation(_r(z2[:]), zp[:], AF.Square)
                rsump = ps.tile([128, TT], F32, tag="bank", name=f"rs{t}")
                nc.tensor.matmul(rsump[:], _r(onesP[:]), _r(z2[:]), start=True, stop=True)
                logp = ps.tile([128, TT], F32, tag="bank", name=f"lg{t}")
                nc.tensor.matmul(logp[:E, :], _r(gm[:]), _r(z2[:]), start=True, stop=True)
                rinv = rt8.tile([E, TT], F32, tag="rinv")
                nc.vector.reciprocal(rinv[:], rsump[:E, :])
                logits = rt8.tile([E, TT], F32, tag="logits")
                nc.vector.tensor_tensor(logits[:], logp[:E, :], rinv[:], ALU.mult)
                expl = rt8.tile([32, TT], F32, tag="expl")
                zbc_e = bass.AP(tensor=zed_d.tensor, offset=0, ap=[[0, 32], [1, TT]])
                nc.sync.dma_start(_r(expl[:]), _r(zbc_e))
                nc.scalar.activation(
                    _r(expl[:E, :]), logits[:], AF.Exp, bias=bia_sb[:], scale=1.0 / TAU
                )
                expl_tiles[t] = expl

            def r_back(t):
                expl = expl_tiles.pop(t)
                esump = ps.tile([128, TT], F32, tag="bank", name=f"es{t}")
                nc.tensor.matmul(
                    esump[:E, :], _r(ones32[:]), _r(expl[:]), start=True, stop=True
                )
                sinv = rt8.tile([E, TT], F32, tag="sinv")
                nc.vector.reciprocal(sinv[:], esump[:E, :])
                nc.vector.tensor_tensor(
                    _r(probs_all[:E, t, :]), expl[:E, :], sinv[:], ALU.mult
                )

            mids_tiles = {}

            def adapter_down(t):
                ht = ht_tiles[t]
                mids = midp.tile([128, MC, TT], F32, tag="mids")
                mids_tiles[t] = mids
                for mc in range(MC):
                    pbp = ps.tile([128, TT], F32, tag="bank", name=f"pb{t}_{mc}")
                    nc.tensor.matmul(
                        pbp[:], _r(sel_sb[:, bass.ts(mc, 128)]), _r(probs_all[:, t, :]),
                        start=True, stop=True,
                    )
                    midps = ps.tile([128, TT], F32, tag="bank", name=f"md{t}_{mc}")
                    for k in range(DK):
                        nc.tensor.matmul(
                            midps[:], _r(wd_sb[:, k, bass.ts(mc, 128)]), _r(ht[:, k, :]),
                            start=(k == 0), stop=(k == DK - 1),
                        )
                    nc.scalar.activation(_r(mids[:, mc, :]), midps[:], AF.Gelu)
                    nc.vector.tensor_tensor(
                        _r(mids[:, mc, :]), mids[:, mc, :], pbp[:], ALU.mult
                    )

            def adapter_up(t, mid_cb=None):
                tsl = bass.ts(t, TT)
                ht = ht_tiles.pop(t)
                mids = mids_tiles.pop(t)
                out_eng = nc.sync if t == NT - 1 else nc.gpsimd
                for dm in range(DK):
                    if dm == 2 and mid_cb is not None:
                        mid_cb()
                    outp = ps.tile([128, TT], F32, tag="bank", name=f"o{t}_{dm}")
                    for kc in range(MC):
                        nc.tensor.matmul(
                            outp[:], _r(wu_sb[:, kc, bass.ts(dm, 128)]), _r(mids[:, kc, :]),
                            start=(kc == 0), stop=(kc == MC - 1),
                        )
                    ysb = yp.tile([128, TT], F32, tag="y")
                    nc.vector.tensor_tensor(ysb[:], outp[:], ht[:, dm, :], ALU.add)
                    out_eng.dma_start(yt_d[bass.ts(dm, 128), tsl], ysb[:])

            def ortho():
                # needs true-fp32 copies of Q/P: writes through an f32r-typed
                # AP round values to tf32, which would swamp the ~1e-6
                # orthogonality residuals.  Borrow an ht-pool slot.
                qp = htp.tile([128, DK, 2 * R], F32, tag="ht")
                nc.sync.dma_start(qp[:, :, :R], q_d.rearrange("(k p) n -> p k n", p=128))
                nc.sync.dma_start(qp[:, :, R:], p_d.rearrange("(k p) n -> p k n", p=128))
                for name, col in (("q", 0), ("p", 1)):
                    w_sb = qp[:, :, col * R : (col + 1) * R]
                    gram = ps.tile([128, TT], F32, tag="bank", name=f"gram_{name}")
                    for k in range(DK):
                        nc.tensor.matmul(
                            gram[:, :R], w_sb[:, k, :], w_sb[:, k, :],
                            start=(k == 0), stop=(k == DK - 1),
                        )
                    dv = rt.tile([128, R], F32, tag="orthd")
                    nc.vector.tensor_tensor(dv[:], gram[:, :R], eye_sb[:], ALU.subtract)
                    nc.vector.tensor_tensor(dv[:], dv[:], dv[:], ALU.mult)
                    nc.vector.tensor_reduce(
                        orth_sb[:, col : col + 1], dv[:], axis=AX.X, op=ALU.add
                    )

            def entropy():
                # batched so Ln costs a single ACT table load, emitted where
                # the PE has adapter work to hide the ACT latency
                for t in range(NT):
                    lp = rt8.tile([E, TT], F32, tag="rinv")
                    nc.scalar.activation(lp[:], probs_all[:E, t, :], AF.Ln)
                    plp = rt8.tile([E, TT], F32, tag="logits")
                    nc.vector.tensor_tensor(plp[:], probs_all[:E, t, :], lp[:], ALU.mult)
                    nc.vector.tensor_reduce(
                        ent_sb[:, t : t + 1], plp[:], axis=AX.X, op=ALU.add
                    )

            # software-pipelined emission order: adapters fill router latency,
            # weight-block DMAs land just before their consumers.
            # ht(0) is DMA'd right after q so the first z matmuls start ASAP.
            ht0 = htp.tile([128, DK, TT], F32, tag="ht")
            for h in range(2):
                nc.sync.dma_start(
                    _r(ht0[:, bass.ts(h, DK // 2), :]),
                    _r(ht_d[bass.ts(h, 512), bass.ts(0, TT)].rearrange("(k p) n -> p k n", p=128)),
                )
            cs = consts()
            sel_sb = cs["sel"]; onesP = cs["onesP"]; ones32 = cs["ones32"]
            probs_all = cs["probs"]; eye_sb = cs["eye"]
            bia_sb = cs["bia"]
            gm = mask_softmax(cs)
            r_front(0, pre_ht=ht0)
            dma_wd()
            dma_wu()
            r_back(0)
            adapter_down(0)
            ortho()
            r_front(1)
            adapter_up(0, mid_cb=lambda: r_back(1))
            adapter_down(1)
            r_front(2)
            adapter_up(1, mid_cb=lambda: r_back(2))
            adapter_down(2)
            r_front(3)
            adapter_up(2, mid_cb=lambda: r_back(3))
            entropy()
            adapter_down(3)
            adapter_up(3)

            nc.sync.dma_start(ent_d[:], ent_sb[:])
            nc.sync.dma_start(orth_d[:], orth_sb[:])

    nc.compile()
    return nc


_NC = None


def _get_nc():
    global _NC
    if _NC is None:
        _NC = _build()
    return _NC


def kernel(x, Q, P, gamma, masks, bias, W_down, W_up, alpha):
    x = np.asarray(x, dtype=np.float32)
    Q = np.ascontiguousarray(np.asarray(Q, dtype=np.float32))
    P = np.ascontiguousarray(np.asarray(P, dtype=np.float32))
    gamma = np.asarray(gamma, dtype=np.float32)
    masks = np.asarray(masks, dtype=np.float32)
    bias = np.asarray(bias, dtype=np.float32)
    W_down = np.asarray(W_down, dtype=np.float32)
    W_up = np.asarray(W_up, dtype=np.float32)
    alpha_f = np.float32(alpha)

    # host-side shard prep (layout transposes)
    xpatch = x[:, 1:, :]                                   # [B, T-1, D]
    wd_host = np.ascontiguousarray(W_down.reshape(EC, D).T)          # [D, EC]
    wu_host = np.ascontiguousarray(
        (W_up.transpose(0, 2, 1).reshape(EC, D)) * alpha_f           # [EC, D]
    )
    mt_host = np.ascontiguousarray(masks.T)                          # [R, E]
    gam_host = np.ascontiguousarray(gamma.reshape(R, 1))
    bia_host = np.ascontiguousarray((bias / np.float32(TAU)).reshape(E, 1))
    sel_host = np.zeros((32, EC), dtype=np.float32)
    for j in range(EC):
        sel_host[j // C, j] = 1.0
    eye_host = np.eye(128, dtype=np.float32)

    in_maps = []
    for c in range(NCORES):
        hTc = np.ascontiguousarray(
            xpatch[c * BPC : (c + 1) * BPC].transpose(2, 0, 1).reshape(D, TOK)
        )
        in_maps.append(
            {
                "ht": hTc,
                "wd": wd_host,
                "wu": wu_host,
                "q": Q,
                "p": P,
                "mt": mt_host,
                "gam": gam_host,
                "bia": bia_host,
                "sel": sel_host,
                "onep": np.ones((128, 128), dtype=np.float32),
                "one32": np.ones((32, E), dtype=np.float32),
                "zed": np.zeros((1, TT), dtype=np.float32),
                "eye": eye_host,
            }
        )

    nc = _get_nc()
    res = run_bass_kernel_spmd(nc, in_maps, list(range(NCORES)))

    # gather / unshard
    y = np.empty((B, T - 1, D), dtype=np.float32)
    ent_total = 0.0
    for c in range(NCORES):
        r = res.results[c]
        y[c * BPC : (c + 1) * BPC] = (
            r["yt"].reshape(D, BPC, T - 1).transpose(1, 2, 0)
        )
        ent_total += r["ent"].astype(np.float64).sum()
    full = np.concatenate([x[:, :1, :], y], axis=1)

    orth = res.results[0]["orth"].astype(np.float64)
    ortho = np.float32(ORTHO_LAMBDA * (orth[:, 0].sum() + orth[:, 1].sum()))
    entropy = np.float32(-ent_total / (B * (T - 1)))
    return full, ortho, entropy
